# revision 1
# baseline (speedup 1.0000x reference)
"""DCNv2 (modulated deformable conv) + BN + SiLU Trainium2 Bass kernel.

Problem: nn_DeformConv_58935541236111
  x[4,256,64,64]: offset/mask conv (3x3, 256->27, +bias) -> clamp/sigmoid ->
  bilinear sampling -> einsum over (C1*KK) with w_dcn -> BN -> SiLU.

Sharding: 8 cores = batch (4) x row-half (2); core computes
out[b, :, 32r:32r+32, :].

Per-core program:
  1. offset conv: 9 shifted matmuls x 2 c-tiles (PE), om psum [27, 2048]
  2. PE-transpose om so pixels land on partitions mod 128
  3. elementwise chain: +bias, clamp, +base, frac via mod, sigmoid ->
     4 bilinear corner weights (bf16) and 4 int16 token indices
  4. wrap16 index layout for dma_gather ([16 part, i//16], replicated x8)
  5. weights: PE-transpose -> DRAM staging in gather order -> partition-
     broadcast -> w_rep[128, 4*2048] per k
  6. per k: two SBUF-source transpose dma_gathers (2 corners each,
     channels-on-partitions bf16), 7-op DVE lerp -> col, 16 matmuls
     accumulating the einsum in PSUM
  7. BN+SiLU in one ACT op per o-tile whose output AP undoes the pixel
     permutation; DMA out.

Pixel permutation: gather free position i = cr*2048 + pi(pix),
  pi(pix) = (p//16)*256 + (pix//128)*16 + (p%16),  p = pix%128
so every wrap/staging DMA moves contiguous 32B runs.

Token grid: 80 rows x 128 cols (pad 8), token = (y+8)*128 + (x+8),
stored [partition=col, stripe=row, 256ch] bf16 for the SBUF-source gather.
"""

import os
import numpy as np
import ml_dtypes

KSTAGE = int(os.environ.get("KSTAGE", "3"))

B, C1, C2, H, W = 4, 256, 256, 64, 64
MAX_OFF = 6.0
BN_EPS = 1e-5

NCORES = 8
HL = 32
P = HL * W              # 2048 pixels / core
GR, GC = 48, 128   # rows h0-8 .. h0+39 (all sampled rows)
PAD = 8
NIDX_H = 2 * P          # indices per half-gather (2 corners)

BF16 = ml_dtypes.bfloat16


def _build_nc():
    import concourse.bacc as bacc
    import concourse.mybir as mybir
    import concourse.tile as tile

    f32 = mybir.dt.float32
    bf16 = mybir.dt.bfloat16
    i16 = mybir.dt.int16
    AF = mybir.ActivationFunctionType
    OP = mybir.AluOpType

    nc = bacc.Bacc("TRN2", target_bir_lowering=False, debug=False)

    x_tok_d = nc.dram_tensor("x_tok", [128, GR, 256], bf16, kind="ExternalInput")
    x_conv_d = nc.dram_tensor("x_conv", [2, 128, 34, 66], bf16, kind="ExternalInput")
    w_om_d = nc.dram_tensor("w_om", [9, 2, 128, 27], bf16, kind="ExternalInput")
    w_dcn_d = nc.dram_tensor("w_dcn", [9, 2, 2, 128, 128], bf16, kind="ExternalInput")
    base_y_d = nc.dram_tensor("base_y", [128, 9, 16], f32, kind="ExternalInput")
    base_x_d = nc.dram_tensor("base_x", [128, 9, 16], f32, kind="ExternalInput")
    bias_y_d = nc.dram_tensor("bias_y", [128, 9, 16], f32, kind="ExternalInput")
    bias_x_d = nc.dram_tensor("bias_x", [128, 9, 16], f32, kind="ExternalInput")
    bias_m_d = nc.dram_tensor("bias_m", [128, 9, 16], f32, kind="ExternalInput")
    ident_d = nc.dram_tensor("ident", [128, 128], bf16, kind="ExternalInput")
    bn_d = nc.dram_tensor("bn", [4, 128, 2], f32, kind="ExternalInput")
    out_d = nc.dram_tensor("out", [2, 128, P], f32, kind="ExternalOutput")
    w_stage_d = nc.dram_tensor("w_stage", [9, 4 * P], bf16)

    with tile.TileContext(nc) as tc:
        with tc.tile_pool(name="persist", bufs=1) as big:
            # ---------- persistent tensors ----------
            x_tok = big.tile([128, GR, 256], bf16)
            nc.sync.dma_start(x_tok[:], x_tok_d[:])
            wd = big.tile([128, 9, 2, 2, 128], bf16)
            nc.sync.dma_start(wd[:], w_dcn_d[:].rearrange("k c o p q -> p k c o q"))
            ident = big.tile([128, 128], bf16)
            nc.scalar.dma_start(ident[:], ident_d[:])
            wrap_rep = big.tile([128, 9, 512], i16)
            bn_s = big.tile([128, 2], f32)
            bn_o = big.tile([128, 2], f32)
            _phase1(nc, tc, mybir, big, x_tok, wd, ident, wrap_rep, bn_s, bn_o,
                    x_conv_d, w_om_d, base_y_d, base_x_d, bias_y_d, bias_x_d,
                    bias_m_d, bn_d, w_stage_d)
            _phase2(nc, tc, mybir, x_tok, wd, ident, wrap_rep, bn_s, bn_o,
                    big, w_stage_d, out_d)

    nc.compile()
    return nc


def _phase1(nc, tc, mybir, big, x_tok, wd, ident, wrap_rep, bn_s, bn_o,
            x_conv_d, w_om_d, base_y_d, base_x_d, bias_y_d, bias_x_d,
            bias_m_d, bn_d, w_stage_d):
    f32 = mybir.dt.float32
    bf16 = mybir.dt.bfloat16
    i16 = mybir.dt.int16
    AF = mybir.ActivationFunctionType
    OP = mybir.AluOpType
    if True:
        with (
            tc.tile_pool(name="chain", bufs=1) as chain,
            tc.tile_pool(name="psum", bufs=1, space="PSUM") as psp,
        ):
            # ---------- static loads (conv inputs first: critical path) ----------
            xc = chain.tile([128, 2, 34, 66], bf16)
            nc.scalar.dma_start(xc[:], x_conv_d[:].rearrange("c p a b -> p c a b"))
            w_om = chain.tile([128, 9, 2, 27], bf16)
            nc.scalar.dma_start(w_om[:], w_om_d[:].rearrange("k c p o -> p k c o"))
            base_y = chain.tile([128, 9, 16], f32, tag="base_y")
            nc.sync.dma_start(base_y[:], base_y_d[:])
            base_x = chain.tile([128, 9, 16], f32, tag="base_x")
            nc.sync.dma_start(base_x[:], base_x_d[:])
            bias_y = chain.tile([128, 9, 16], f32, tag="bias_y")
            nc.sync.dma_start(bias_y[:], bias_y_d[:])
            bias_x = chain.tile([128, 9, 16], f32, tag="bias_x")
            nc.sync.dma_start(bias_x[:], bias_x_d[:])
            bias_m = chain.tile([128, 9, 16], f32, tag="bias_m")
            nc.sync.dma_start(bias_m[:], bias_m_d[:])
            bn_in = chain.tile([128, 4, 2], f32, tag="bn_in")
            nc.sync.dma_start(bn_in[:], bn_d[:].rearrange("a p b -> p a b"))

            # ---------- BN constants on device ----------
            # bn_in[:, 0]=gamma, 1=beta, 2=mean, 3=var  (each [128, 2])
            tvar = chain.tile([128, 2], f32, tag="tvar")
            nc.vector.tensor_scalar(tvar[:], bn_in[:, 3], BN_EPS, None, OP.add)
            nc.scalar.sqrt(tvar[:], tvar[:])
            nc.vector.reciprocal(tvar[:], tvar[:])
            nc.vector.tensor_tensor(bn_s[:], bn_in[:, 0], tvar[:], OP.mult)   # inv
            nc.vector.tensor_tensor(bn_o[:], bn_in[:, 2], bn_s[:], OP.mult)   # mean*inv
            nc.vector.tensor_tensor(bn_o[:], bn_in[:, 1], bn_o[:], OP.subtract)  # beta-mean*inv

            # ---------- 1. offset conv ----------
            om_ps = psp.tile([27, P], f32, tag="pa")
            for ky in range(3):
                for kx in range(3):
                    k = ky * 3 + kx
                    for ct in range(2):
                        for n in range(4):
                            nc.tensor.matmul(
                                om_ps[:, n * 512:(n + 1) * 512],
                                w_om[:, k, ct],
                                xc[:, ct, ky + n * 8: ky + n * 8 + 8, kx: kx + 64],
                                start=(k == 0 and ct == 0),
                                stop=(k == 8 and ct == 1),
                            )
            om_sb = chain.tile([27, P], bf16, tag="om_sb")
            nc.scalar.copy(om_sb[:], om_ps[:])

            # ---------- 2. PE transpose om -> [128, 16, 27] ----------
            omT_ps = psp.tile([128, 16 * 28], bf16, tag="pb")
            for ch in range(16):
                nc.tensor.transpose(
                    omT_ps[:, ch * 28:ch * 28 + 27],
                    om_sb[:, ch * 128:(ch + 1) * 128],
                    ident[:27, :27],
                )
            omT = chain.tile([128, 16, 27], f32, tag="omT")
            nc.scalar.copy(
                omT[:],
                omT_ps[:].rearrange("p (a b) -> p a b", a=16)[:, :, 0:27],
            )

            # ---------- 3. elementwise chain [128, 9, 16] ----------
            def ct_(name):
                return chain.tile([128, 9, 16], f32, tag=name, name=name)

            dy = ct_("dy"); dx = ct_("dx"); mm = ct_("mm")
            omT_r = omT[:].rearrange("p c o -> p o c")
            nc.vector.tensor_copy(dy[:], omT_r[:, 0:18:2, :])
            nc.vector.tensor_copy(dx[:], omT_r[:, 1:18:2, :])
            nc.vector.tensor_copy(mm[:], omT_r[:, 18:27, :])

            t0 = ct_("t0"); t1 = ct_("t1")
            nc.vector.tensor_tensor(dy[:], dy[:], bias_y[:], OP.add)
            nc.vector.tensor_tensor(dx[:], dx[:], bias_x[:], OP.add)
            nc.vector.tensor_tensor(mm[:], mm[:], bias_m[:], OP.add)
            nc.vector.tensor_scalar(t0[:], dy[:], MAX_OFF, -MAX_OFF, OP.min, OP.max)
            nc.vector.tensor_scalar(t1[:], dx[:], MAX_OFF, -MAX_OFF, OP.min, OP.max)
            pys = ct_("pys"); pxs = ct_("pxs")
            nc.vector.tensor_tensor(pys[:], t0[:], base_y[:], OP.add)
            nc.vector.tensor_tensor(pxs[:], t1[:], base_x[:], OP.add)
            ly = ct_("ly"); lx = ct_("lx")
            y0 = ct_("y0"); x0 = ct_("x0")
            iy = chain.tile([128, 9, 16], mybir.dt.int32, tag="iy", name="iy")
            ix = chain.tile([128, 9, 16], mybir.dt.int32, tag="ix", name="ix")
            # floor(pys) robust to converter rounding mode: y0 = cvt(pys);
            # y0 -= (y0 > pys)
            nc.vector.tensor_copy(iy[:], pys[:])
            nc.vector.tensor_copy(y0[:], iy[:])
            nc.vector.tensor_tensor(t0[:], y0[:], pys[:], OP.is_gt)
            nc.vector.tensor_tensor(y0[:], y0[:], t0[:], OP.subtract)
            nc.vector.tensor_tensor(ly[:], pys[:], y0[:], OP.subtract)
            nc.vector.tensor_copy(ix[:], pxs[:])
            nc.vector.tensor_copy(x0[:], ix[:])
            nc.vector.tensor_tensor(t1[:], x0[:], pxs[:], OP.is_gt)
            nc.vector.tensor_tensor(x0[:], x0[:], t1[:], OP.subtract)
            nc.vector.tensor_tensor(lx[:], pxs[:], x0[:], OP.subtract)
            # indices first: the gather critical path starts here
            idxf = ct_("idxf")
            nc.vector.tensor_scalar(t0[:], y0[:], 128.0, None, OP.mult)
            nc.vector.tensor_tensor(idxf[:], t0[:], x0[:], OP.add)
            idx_all = chain.tile([128, 4, 9, 16], i16, tag="idx_all")
            for cr, off in enumerate([0.0, 1.0, 128.0, 129.0]):
                nc.vector.tensor_scalar(t1[:], idxf[:], off, None, OP.add)
                nc.vector.tensor_copy(idx_all[:, cr], t1[:])
            msk = ct_("msk")
            nc.scalar.activation(msk[:], mm[:], AF.Sigmoid)
            oly = ct_("oly"); olx = ct_("olx")
            nc.vector.tensor_scalar(oly[:], ly[:], -1.0, 1.0, OP.mult, OP.add)
            nc.vector.tensor_scalar(olx[:], lx[:], -1.0, 1.0, OP.mult, OP.add)
            wyt = ct_("wyt"); wyb = ct_("wyb")
            nc.vector.tensor_tensor(wyt[:], oly[:], msk[:], OP.mult)
            nc.vector.tensor_tensor(wyb[:], ly[:], msk[:], OP.mult)
            wf = chain.tile([128, 4, 9, 16], bf16, tag="wf")
            nc.vector.tensor_tensor(wf[:, 0], wyt[:], olx[:], OP.mult)
            nc.vector.tensor_tensor(wf[:, 1], wyt[:], lx[:], OP.mult)
            nc.vector.tensor_tensor(wf[:, 2], wyb[:], olx[:], OP.mult)
            nc.vector.tensor_tensor(wf[:, 3], wyb[:], lx[:], OP.mult)

            # ---------- 4. wrap16 indices ----------
            # wrap16[q, k, cr*128 + ph*16 + fl] = idx_all[ph*16+q, cr, k, fl]
            wrap16 = chain.tile([16, 9, 4, 8, 16], i16, tag="wrap16")
            for ph in range(8):
                for cr in range(4):
                    eng = nc.sync if (ph * 4 + cr) % 2 == 0 else nc.scalar
                    eng.dma_start(
                        wrap16[:, :, cr, ph, :],
                        idx_all[ph * 16:(ph + 1) * 16, cr],
                    )
            for g8 in range(8):
                eng = nc.sync if g8 % 2 == 0 else nc.scalar
                eng.dma_start(
                    wrap_rep[g8 * 16:(g8 + 1) * 16],
                    wrap16[:].rearrange("q k cr ph fl -> q k (cr ph fl)"),
                )

            # ---------- 5. weight transpose + staging ----------
            for k in range(9):
                wT_ps = psp.tile([16, 4 * 128], bf16, tag="pa", name=f"wT_ps{k}")
                for cr in range(4):
                    nc.tensor.transpose(
                        wT_ps[:, cr * 128:(cr + 1) * 128],
                        wf[:, cr, k, :],
                        ident[:],
                    )
                wT = chain.tile([16, 4, 8, 16], bf16, tag="wT", name=f"wT{k}")
                nc.scalar.copy(wT[:].rearrange("p a b c -> p (a b c)"), wT_ps[:])
                eng = nc.sync if k % 2 == 0 else nc.scalar
                eng.dma_start(
                    w_stage_d[k].rearrange("(cr ph fl pl) -> fl cr ph pl", cr=4, ph=8, fl=16),
                    wT[:],
                )


def _phase2(nc, tc, mybir, x_tok, wd, ident, wrap_rep, bn_s, bn_o,
            big2_outer, w_stage_d, out_d):
    f32 = mybir.dt.float32
    bf16 = mybir.dt.bfloat16
    AF = mybir.ActivationFunctionType
    OP = mybir.AluOpType
    if True:
        # ---------- 6. main k-loop ----------
        with (
            tc.tile_pool(name="big2", bufs=1) as big2,
            tc.tile_pool(name="gbuf", bufs=2) as gbuf,
            tc.tile_pool(name="wrepp", bufs=2) as wrepp,
            tc.tile_pool(name="colp", bufs=2) as colp,
            tc.tile_pool(name="tmp", bufs=1) as tmpp,
            tc.tile_pool(name="psum2", bufs=1, space="PSUM") as psp2,
        ):
            out_ps = [psp2.tile([128, P], f32, tag=f"o{ot}", name=f"out_ps{ot}") for ot in range(2)]
            for k in range(9):
                col = colp.tile([128, 2, P], bf16, tag="col", name=f"col{k}")
                if KSTAGE < 1:
                    nc.vector.memset(col[:], 0.25)
                    _emit_einsum(nc, col, wd, out_ps, k)
                    continue
                w_rep = wrepp.tile([128, 4 * P], bf16, tag="w_rep", name=f"w_rep{k}")
                nc.sync.dma_start(
                    w_rep[:],
                    w_stage_d[k].partition_broadcast(128),
                )
                if KSTAGE < 2:
                    nc.vector.tensor_copy(col[:].rearrange("p a b -> p (a b)"), w_rep[:, 0:2 * P])
                    _emit_einsum(nc, col, wd, out_ps, k)
                    continue
                ghs = []
                for half in range(2):
                    gh = gbuf.tile([128, 2, NIDX_H], bf16, tag=f"g{half}", name=f"g{k}_{half}")
                    nc.gpsimd.dma_gather(
                        gh[:],
                        x_tok[:].rearrange("p r c -> p (r c)"),
                        wrap_rep[:, k, half * 256:(half + 1) * 256],
                        NIDX_H,
                        NIDX_H,
                        256,
                        transpose=True,
                        sbuf_tokens_per_rank=128,
                        sbuf_free_dim_per_rank=512,
                        single_packet=False,
                    )
                    ghs.append(gh)
                if KSTAGE < 3:
                    nc.vector.tensor_copy(col[:, 0], ghs[0][:, 0, 0:P])
                    nc.vector.tensor_copy(col[:, 1], ghs[1][:, 1, 0:P])
                    _emit_einsum(nc, col, wd, out_ps, k)
                    continue
                ta = tmpp.tile([128, P], bf16, tag="ta", name=f"ta{k}")
                for ctile in range(2):
                    v00 = ghs[0][:, ctile, 0:P]
                    v01 = ghs[0][:, ctile, P:2 * P]
                    v10 = ghs[1][:, ctile, 0:P]
                    v11 = ghs[1][:, ctile, P:2 * P]
                    w00 = w_rep[:, 0:P]
                    w01 = w_rep[:, P:2 * P]
                    w10 = w_rep[:, 2 * P:3 * P]
                    w11 = w_rep[:, 3 * P:4 * P]
                    cc = col[:, ctile]
                    nc.vector.tensor_tensor(cc, v00, w00, OP.mult)
                    nc.vector.tensor_tensor(ta[:], v01, w01, OP.mult)
                    nc.vector.tensor_tensor(cc, cc, ta[:], OP.add)
                    nc.vector.tensor_tensor(ta[:], v10, w10, OP.mult)
                    nc.vector.tensor_tensor(cc, cc, ta[:], OP.add)
                    nc.vector.tensor_tensor(ta[:], v11, w11, OP.mult)
                    nc.vector.tensor_tensor(cc, cc, ta[:], OP.add)
                _emit_einsum(nc, col, wd, out_ps, k)

            # ---------- 7. BN + SiLU + unpermute + store ----------
            for ot in range(2):
                yv = big2.tile([128, P], f32, tag="yv", name=f"yv{ot}")
                sg = big2.tile([128, P], f32, tag="sg", name=f"sg{ot}")
                o_sb = big2.tile([128, P], f32, tag=f"osb{ot}", name=f"o_sb{ot}")
                nc.vector.tensor_scalar(
                    yv[:], out_ps[ot][:],
                    bn_s[:, ot:ot + 1], bn_o[:, ot:ot + 1],
                    OP.mult, OP.add,
                )
                nc.scalar.activation(sg[:], yv[:], AF.Sigmoid)
                nc.vector.tensor_tensor(
                    o_sb[:].rearrange("p (c b a) -> p c b a", c=16, b=8),
                    yv[:].rearrange("p (b c a) -> p c b a", b=8, c=16),
                    sg[:].rearrange("p (b c a) -> p c b a", b=8, c=16),
                    OP.mult,
                )
                nc.sync.dma_start(out_d[ot], o_sb[:])


def _emit_einsum(nc, col, wd, out_ps, k):
    for ctile in range(2):
        for ot in range(2):
            for n in range(4):
                nc.tensor.matmul(
                    out_ps[ot][:, n * 512:(n + 1) * 512],
                    wd[:, k, ctile, ot],
                    col[:, ctile, n * 512:(n + 1) * 512],
                    start=(k == 0 and ctile == 0),
                    stop=(k == 8 and ctile == 1),
                )


def _prep_core_inputs(inputs, b, r):
    x = np.asarray(inputs["x"])
    w_om = np.asarray(inputs["w_om"])
    b_om = np.asarray(inputs["b_om"])
    w_dcn = np.asarray(inputs["w_dcn"])
    h0 = HL * r

    xp = np.zeros((GR, GC, 256), dtype=BF16)
    y_lo, y_hi = max(0, h0 - PAD), min(H, h0 + HL + PAD)
    xp[y_lo - (h0 - PAD):y_hi - (h0 - PAD), PAD:PAD + W, :] = (
        x[b][:, y_lo:y_hi, :].transpose(1, 2, 0).astype(BF16)
    )
    x_tok = np.ascontiguousarray(xp.swapaxes(0, 1))          # [128, 48, 256]

    xcv = np.zeros((256, 34, 66), dtype=BF16)
    r_lo, r_hi = max(0, h0 - 1), min(H, h0 + 33)
    xcv[:, r_lo - (h0 - 1):r_hi - (h0 - 1), 1:65] = x[b][:, r_lo:r_hi, :].astype(BF16)
    x_conv = np.ascontiguousarray(xcv.reshape(2, 128, 34, 66))

    wl = np.zeros((9, 2, 128, 27), dtype=BF16)
    for ky in range(3):
        for kx in range(3):
            k = ky * 3 + kx
            for ctile in range(2):
                wl[k, ctile] = w_om[:, ctile * 128:(ctile + 1) * 128, ky, kx].T.astype(BF16)

    wdl = np.zeros((9, 2, 2, 128, 128), dtype=BF16)
    wr = w_dcn.reshape(C2, C1, 9)
    for k in range(9):
        for ctile in range(2):
            for ot in range(2):
                wdl[k, ctile, ot] = wr[ot * 128:(ot + 1) * 128,
                                       ctile * 128:(ctile + 1) * 128, k].T.astype(BF16)

    p_ = np.arange(128)[:, None, None]
    k_ = np.arange(9)[None, :, None]
    fl = np.arange(16)[None, None, :]
    pix = fl * 128 + p_                       # [128, 1, 16] + broadcast
    h_loc = pix // W
    w_pix = pix % W
    ky_ = k_ // 3
    kx_ = k_ % 3
    base_y = np.broadcast_to(h_loc + ky_ - 1 + PAD, (128, 9, 16)).astype(np.float32)
    base_x = np.broadcast_to(w_pix + kx_ - 1 + PAD, (128, 9, 16)).astype(np.float32)
    bias_y = np.broadcast_to(b_om[0:18:2][None, :, None], (128, 9, 16)).astype(np.float32)
    bias_x = np.broadcast_to(b_om[1:18:2][None, :, None], (128, 9, 16)).astype(np.float32)
    bias_m = np.broadcast_to(b_om[18:27][None, :, None], (128, 9, 16)).astype(np.float32)

    bn = np.stack([
        np.asarray(inputs["bn_gamma"]).reshape(2, 128).T,
        np.asarray(inputs["bn_beta"]).reshape(2, 128).T,
        np.asarray(inputs["bn_mean"]).reshape(2, 128).T,
        np.asarray(inputs["bn_var"]).reshape(2, 128).T,
    ], axis=0).astype(np.float32)             # [4, 128, 2]

    return {
        "x_tok": x_tok,
        "x_conv": x_conv,
        "w_om": wl,
        "w_dcn": wdl,
        "base_y": np.ascontiguousarray(base_y),
        "base_x": np.ascontiguousarray(base_x),
        "bias_y": np.ascontiguousarray(bias_y),
        "bias_x": np.ascontiguousarray(bias_x),
        "bias_m": np.ascontiguousarray(bias_m),
        "ident": np.eye(128, dtype=BF16),
        "bn": np.ascontiguousarray(bn),
    }


_NC_CACHE = {}


def _get_nc():
    if "nc" not in _NC_CACHE:
        _NC_CACHE["nc"] = _build_nc()
    return _NC_CACHE["nc"]


def _assemble(results):
    out = np.zeros((B, C2, H, W), dtype=np.float32)
    for c in range(NCORES):
        b, r = c // 2, c % 2
        o = np.asarray(results[c]["out"])     # [2, 128, 2048]
        for ot in range(2):
            out[b, ot * 128:(ot + 1) * 128, HL * r:HL * (r + 1), :] = (
                o[ot].reshape(128, HL, W).astype(np.float32)
            )
    return out


def _run(inputs, trace=False):
    from concourse.bass_utils import run_bass_kernel_spmd
    nc = _get_nc()
    in_maps = [_prep_core_inputs(inputs, c // 2, c % 2) for c in range(NCORES)]
    res = run_bass_kernel_spmd(nc, in_maps, list(range(NCORES)), trace=trace)
    return _assemble(res.results), res


def kernel(**inputs):
    out, _ = _run(inputs, trace=False)
    return out

